# revision 3
# baseline (speedup 1.0000x reference)
"""Trainium2 Bass kernel for nn_Attention (B=4, N=2048, D=1024, H=16, Hd=64).

Sharding: 8 cores = 4 batches x 2 head-groups. Core c handles batch c//2 and
heads [ (c%2)*8, (c%2)*8+8 ).  Each core computes qkv projections for its
heads, attention, and a partial output projection (contraction over its 512
head-dims of W_proj). Host sums the two partials per batch and adds b_proj.

Per-core kernel (all matmuls bf16 with fp32 PSUM accumulation):
  - qkT[f, t]  = sum_d Wqk[d, f] * xT[d, t]     (Q^T/K^T per head, [64, 2048])
  - v[t, f]    = sum_d xT[d, t] * Wv[d, f]       ([2048, 512], keys-major)
  - per head pair (2 heads packed in PE row/col groups):
      S^T[k, q] = sum_d K^T[d, k] Q^T[d, q]      (keys on partitions)
      E = exp(S^T / 8)   (ScalarE, bf16 out)
      U^T[hd, q] += sum_k V[k, hd] E[k, q]       (PSUM accumulate over key tiles)
      Eacc += E (VectorE);  sums = partition_all_reduce(Eacc)  (GpSimd)
      Uhat = U^T * (1/sums)                      (normalize during PSUM drain)
  - y[q, e] = sum_hd Uhat[hd, q] Wp[hd, e]       (partial; host adds pair+bias)
"""

import os
import sys
import types

import numpy as np

# --- environment bootstrap (grading env == dev env: axon-tunneled trn2) ----
for _p in ("/opt/trn_rl_repo", "/root/.axon_site/_ro/trn_rl_repo"):
    if _p not in sys.path and os.path.isdir(_p):
        sys.path.append(_p)

import ml_dtypes  # noqa: E402

BF16 = ml_dtypes.bfloat16


def _install_ntff_shim():
    """antenv.axon_hooks is missing on this image; provide it and register the
    ctypes NTFF hook so trace=True can report HW exec time."""
    if "antenv.axon_hooks" in sys.modules:
        return
    mod = types.ModuleType("antenv.axon_hooks")
    mod._hook = None
    mod.set_axon_ntff_profile_hook = lambda h: setattr(mod, "_hook", h)
    mod.get_axon_ntff_profile_hook = lambda: mod._hook
    sys.modules["antenv.axon_hooks"] = mod
    try:
        import antenv

        antenv.axon_hooks = mod
    except ImportError:
        pass
    try:
        from trn_agent_boot.trn_boot import _ntff_profile_via_ctypes

        hook = _ntff_profile_via_ctypes("/opt/axon/libaxon_pjrt.so")
        if hook is not None:
            mod.set_axon_ntff_profile_hook(hook)
    except Exception:
        pass


_install_ntff_shim()

import concourse.bacc as bacc  # noqa: E402
import concourse.bass as bass  # noqa: E402
import concourse.tile as tile  # noqa: E402
import concourse.bass_isa as bass_isa  # noqa: E402
from concourse import mybir  # noqa: E402
import concourse.bass_utils as bass_utils  # noqa: E402

# no S3 in the container; keep NTFF artifacts local
bass_utils.upload_artifacts = lambda tmpdir: tmpdir

F32 = mybir.dt.float32
BF = mybir.dt.bfloat16
EXP = mybir.ActivationFunctionType.Exp

N_CORES = 8
NT = 2048  # tokens
D = 1024  # d_model
NH_LOC = 8  # heads per core
HD = 64  # head dim
SCALE = HD**-0.5


def _body(tc: "tile.TileContext", ctx, y, xT, wqk, wv, wp):
    nc = tc.nc

    wpool = ctx.enter_context(tc.tile_pool(name="wpool", bufs=1))
    qkpool = ctx.enter_context(tc.tile_pool(name="qkpool", bufs=1))
    vpool = ctx.enter_context(tc.tile_pool(name="vpool", bufs=1))
    upool = ctx.enter_context(tc.tile_pool(name="upool", bufs=1))
    epool = ctx.enter_context(tc.tile_pool(name="epool", bufs=4))
    eaccpool = ctx.enter_context(tc.tile_pool(name="eaccpool", bufs=3))
    spool = ctx.enter_context(tc.tile_pool(name="spool", bufs=2))
    rpool = ctx.enter_context(tc.tile_pool(name="rpool", bufs=1))
    opool = ctx.enter_context(tc.tile_pool(name="opool", bufs=2))
    psb = ctx.enter_context(tc.tile_pool(name="psb", bufs=2, space="PSUM"))
    psu = ctx.enter_context(tc.tile_pool(name="psu", bufs=1, space="PSUM"))

    # ---- persistent SBUF tensors -----------------------------------------
    xT_sb = []
    for i in range(8):
        t = wpool.tile([128, NT], BF, tag=f"xT{i}", name=f"xT{i}")
        nc.sync.dma_start(out=t, in_=xT[i * 128 : (i + 1) * 128, :])
        xT_sb.append(t)
    wqk_sb = []
    for i in range(8):
        t = wpool.tile([128, 1024], BF, tag=f"wqk{i}", name=f"wqk{i}")
        nc.sync.dma_start(out=t, in_=wqk[i * 128 : (i + 1) * 128, :])
        wqk_sb.append(t)
    wv_sb = []
    for i in range(8):
        t = wpool.tile([128, 512], BF, tag=f"wv{i}", name=f"wv{i}")
        nc.sync.dma_start(out=t, in_=wv[i * 128 : (i + 1) * 128, :])
        wv_sb.append(t)
    wp_sb = []
    for i in range(4):
        t = wpool.tile([128, 1024], BF, tag=f"wp{i}", name=f"wp{i}")
        nc.sync.dma_start(out=t, in_=wp[i * 128 : (i + 1) * 128, :])
        wp_sb.append(t)

    qkT = [qkpool.tile([128, NT], BF, tag=f"qkT{f}", name=f"qkT{f}") for f in range(8)]
    v_sb = [vpool.tile([128, 512], BF, tag=f"v{t}", name=f"v{t}") for t in range(16)]
    uhat = [upool.tile([128, NT], BF, tag=f"uh{p}", name=f"uh{p}") for p in range(4)]

    # ---- qkv projections --------------------------------------------------
    def qk_unit(f):
        # qkT[f][ff, t] = sum_d wqk[d, f*128+ff] * xT[d, t]
        for ts in range(2):
            ps = psb.tile([128, 1024], F32, tag="psb", name=f"qk_ps{f}_{ts}")
            for d in range(8):
                for s in range(2):
                    nc.tensor.matmul(
                        ps[:, s * 512 : (s + 1) * 512],
                        wqk_sb[d][:, f * 128 : (f + 1) * 128],
                        xT_sb[d][:, ts * 1024 + s * 512 : ts * 1024 + (s + 1) * 512],
                        start=(d == 0),
                        stop=(d == 7),
                    )
            nc.vector.tensor_copy(out=qkT[f][:, ts * 1024 : (ts + 1) * 1024], in_=ps[:])

    def v_unit(t):
        # v[t*128+tt, f] = sum_d xT[d, t*128+tt] * wv[d, f]; psum shares psu pool
        ps = psu.tile([128, 512], F32, tag="ut", name=f"v_ps{t}")
        for d in range(8):
            nc.tensor.matmul(
                ps[:, :],
                xT_sb[d][:, t * 128 : (t + 1) * 128],
                wv_sb[d][:, :],
                start=(d == 0),
                stop=(d == 7),
            )
        nc.vector.tensor_copy(out=v_sb[t], in_=ps[:])

    # ---- attention for one pair of heads (A=2p at rows 0:64, B at 64:128) -
    def attention_pair(p):
        A, B = 2 * p, 2 * p + 1
        qA = qkT[p][0:64, :]
        qB = qkT[p][64:128, :]
        kA = qkT[4 + p][0:64, :]
        kB = qkT[4 + p][64:128, :]
        ut = psu.tile([128, NT], F32, tag="ut", name=f"ut{p}")
        eaccA = eaccpool.tile([128, NT], BF, tag="eacc", name=f"eaccA{p}")
        eaccB = eaccpool.tile([128, NT], BF, tag="eacc", name=f"eaccB{p}")
        for kt in range(16):
            ksl = slice(kt * 128, (kt + 1) * 128)
            eA = epool.tile([128, NT], BF, tag="e", name=f"eA{p}_{kt}")
            eB = epool.tile([128, NT], BF, tag="e", name=f"eB{p}_{kt}")
            for qh in range(2):
                stA = psb.tile([128, 1024], F32, tag="psb", name=f"stA{p}_{kt}_{qh}")
                stB = psb.tile([128, 1024], F32, tag="psb", name=f"stB{p}_{kt}_{qh}")
                for s in range(2):
                    q0 = qh * 1024 + s * 512
                    nc.tensor.matmul(
                        stA[:, s * 512 : (s + 1) * 512],
                        kA[:, ksl],
                        qA[:, q0 : q0 + 512],
                        start=True,
                        stop=True,
                    )
                    nc.tensor.matmul(
                        stB[:, s * 512 : (s + 1) * 512],
                        kB[:, ksl],
                        qB[:, q0 : q0 + 512],
                        start=True,
                        stop=True,
                    )
                nc.scalar.activation(
                    out=eA[:, qh * 1024 : (qh + 1) * 1024], in_=stA[:], func=EXP, scale=SCALE
                )
                nc.scalar.activation(
                    out=eB[:, qh * 1024 : (qh + 1) * 1024], in_=stB[:], func=EXP, scale=SCALE
                )
            # accumulate exp over key tiles (for softmax denominators)
            if kt == 0:
                nc.vector.tensor_copy(out=eaccA, in_=eA)
                nc.vector.tensor_copy(out=eaccB, in_=eB)
            else:
                nc.vector.tensor_add(out=eaccA, in0=eaccA, in1=eA)
                nc.vector.tensor_add(out=eaccB, in0=eaccB, in1=eB)
            # U^T += V^T E  (col-group packed: A -> rows 0:64, B -> rows 64:128)
            for s in range(4):
                ssl = slice(s * 512, (s + 1) * 512)
                nc.tensor.matmul(
                    ut[0:64, ssl],
                    v_sb[kt][:, A * 64 : (A + 1) * 64],
                    eA[:, ssl],
                    start=(kt == 0),
                    stop=(kt == 15),
                )
                nc.tensor.matmul(
                    ut[64:128, ssl],
                    v_sb[kt][:, B * 64 : (B + 1) * 64],
                    eB[:, ssl],
                    start=(kt == 0),
                    stop=(kt == 15),
                )
        # softmax denominators: all-reduce over the 128 key partitions (GpSimd),
        # then reciprocal; rows 0:64 <- head A, rows 64:128 <- head B.
        sumsA = spool.tile([128, NT], F32, tag="sums", name=f"sumsA{p}")
        sumsB = spool.tile([128, NT], F32, tag="sums", name=f"sumsB{p}")
        nc.gpsimd.partition_all_reduce(
            out_ap=sumsA[:], in_ap=eaccA[:], channels=128, reduce_op=bass_isa.ReduceOp.add
        )
        nc.gpsimd.partition_all_reduce(
            out_ap=sumsB[:], in_ap=eaccB[:], channels=128, reduce_op=bass_isa.ReduceOp.add
        )
        rec = rpool.tile([128, NT], F32, tag="rec", name=f"rec{p}")
        nc.vector.reciprocal(out=rec[0:64, :], in_=sumsA[0:64, :])
        nc.vector.reciprocal(out=rec[64:128, :], in_=sumsB[64:128, :])
        # normalize while draining PSUM -> bf16 SBUF
        for qh in range(2):
            qsl = slice(qh * 1024, (qh + 1) * 1024)
            nc.vector.tensor_mul(uhat[p][:, qsl], ut[:, qsl], rec[:, qsl])

    # ---- output projection (partial over local 512 head dims) ------------
    def proj_unit(qt):
        pj = psb.tile([128, 1024], F32, tag="psb", name=f"pj{qt}")
        for es in range(2):
            for c in range(4):
                nc.tensor.matmul(
                    pj[:, es * 512 : (es + 1) * 512],
                    uhat[c][:, qt * 128 : (qt + 1) * 128],
                    wp_sb[c][:, es * 512 : (es + 1) * 512],
                    start=(c == 0),
                    stop=(c == 3),
                )
        ot = opool.tile([128, 1024], F32, tag="out", name=f"ot{qt}")
        nc.vector.tensor_copy(out=ot, in_=pj[:])
        nc.sync.dma_start(out=y[qt * 128 : (qt + 1) * 128, :], in_=ot)

    # ---- schedule ---------------------------------------------------------
    qk_unit(0)
    qk_unit(4)
    for t in range(16):
        v_unit(t)
    qk_unit(1)
    qk_unit(5)
    qk_unit(2)
    qk_unit(6)
    qk_unit(3)
    qk_unit(7)
    for p in range(4):
        attention_pair(p)
    for qt in range(16):
        proj_unit(qt)


_NC_CACHE = {}


def _build_nc():
    if "nc" in _NC_CACHE:
        return _NC_CACHE["nc"]
    nc = bacc.Bacc("TRN2", target_bir_lowering=False, debug=False, num_devices=N_CORES)
    xT = nc.dram_tensor("xT", [D, NT], BF, kind="ExternalInput").ap()
    wqk = nc.dram_tensor("wqk", [D, 1024], BF, kind="ExternalInput").ap()
    wv = nc.dram_tensor("wv", [D, 512], BF, kind="ExternalInput").ap()
    wp = nc.dram_tensor("wp", [512, 1024], BF, kind="ExternalInput").ap()
    y = nc.dram_tensor("y", [NT, 1024], F32, kind="ExternalOutput").ap()
    from contextlib import ExitStack

    with tile.TileContext(nc) as tc, ExitStack() as ctx:
        _body(tc, ctx, y, xT, wqk, wv, wp)
    nc.compile()
    _NC_CACHE["nc"] = nc
    return nc


def _prepare_in_maps(x, W_qkv, W_proj):
    x = np.asarray(x, dtype=np.float32)
    W_qkv = np.asarray(W_qkv, dtype=np.float32)
    W_proj = np.asarray(W_proj, dtype=np.float32)
    in_maps = []
    for c in range(N_CORES):
        b, hg = divmod(c, 2)
        cs = slice(hg * 512, (hg + 1) * 512)
        xTc = np.ascontiguousarray(x[b].T).astype(BF16)
        wqk = np.ascontiguousarray(
            np.concatenate([W_qkv[:, 0:1024][:, cs], W_qkv[:, 1024:2048][:, cs]], axis=1)
        ).astype(BF16)
        wv = np.ascontiguousarray(W_qkv[:, 2048:3072][:, cs]).astype(BF16)
        wp = np.ascontiguousarray(W_proj[cs, :]).astype(BF16)
        in_maps.append({"xT": xTc, "wqk": wqk, "wv": wv, "wp": wp})
    return in_maps


def _run(x, W_qkv, W_proj, b_proj, trace=False):
    nc = _build_nc()
    in_maps = _prepare_in_maps(x, W_qkv, W_proj)
    res = bass_utils.run_bass_kernel_spmd(
        nc, in_maps, core_ids=list(range(N_CORES)), trace=trace
    )
    b_proj = np.asarray(b_proj, dtype=np.float32)
    y = np.empty((4, NT, D), dtype=np.float32)
    for b in range(4):
        y[b] = res.results[2 * b]["y"] + res.results[2 * b + 1]["y"] + b_proj[None, :]
    return y, res


def kernel(x, W_qkv, W_proj, b_proj):
    y, _ = _run(x, W_qkv, W_proj, b_proj, trace=False)
    return y
